# revision 13
# baseline (speedup 1.0000x reference)
"""Sliding-window GQA self-attention (B=2,T=2048,E=2048,H=16,KV=4,D=128,W=512)
on 8 Trainium2 NeuronCores.

Sharding: sequence-parallel. Core c owns 512 query rows (batch c//4, quarter
c%4) and receives a 512-row key/value halo (zero-padded before the sequence
start; padded keys contribute exactly exp(0)=1 to the softmax denominator,
which is subtracted out via a precomputed correction vector).

On-chip dataflow (per core):
  q/k projections run in fp8e4 with DoubleRow perf mode (K=256 per matmul,
  measured ~1.44x bf16); weights are pre-scaled by 128 into fp8 range and the
  descale is folded into the fp32 RoPE tables (along with 1/sqrt(D) for q).
  The fp8 quantization noise is strongly damped by the near-uniform softmax
  (scores std ~0.023): measured cost ~1e-3 rel err. v projection and
  everything downstream stays bf16 (fp8 there costs 2e-2+).

  DVE ops cost ~550ns each regardless of size, so the kernel minimizes DVE
  op count and spreads elementwise work across engines: RoPE is 2 DVE
  multiplies on all 128 partitions (stacked [cos;cos], [+sin;-sin] tables)
  plus 2 GpSimd adds that un-swap the halves via partition-offset reads;
  PSUM evacuations ride ScalarE; the denominator correction subtract rides
  GpSimd. Scores are computed transposed (scoresT[k, q], 4 query heads of a
  kv group batched into one N=512 matmul) -> exp on ScalarE (no max
  subtraction: |scores| < 0.12) -> sliding-window masks applied
  multiplicatively post-exp. Softmax denominator: the 5 prob blocks per
  (group, query-block) are summed with 4 bf16 DVE adds, then a single
  ones-vector matmul. Emission order keeps the tensor engine fed: scores run
  ahead of attn@v, the last 3 v-projection blocks fill the attention-qb0
  window, and the output projection of query block qb-1 interleaves
  per-group with the attention of qb.
"""

import numpy as np
import ml_dtypes

import concourse.bass as bass
import concourse.bacc as bacc
import concourse.mybir as mybir
import concourse.tile as tile
from concourse.bass_utils import run_bass_kernel_spmd

BF16 = ml_dtypes.bfloat16
FP8 = ml_dtypes.float8_e4m3fn

B, T, E = 2, 2048, 2048
H, KV, D = 16, 4, 128
NREP = H // KV  # 4 query heads per kv head
WINDOW = 512
THETA = 10000.0
W_SCALE = 128.0  # fp8 pre-scale on Wq/Wk; descale folded into rope tables

NCORES = 8
Q = 512          # owned query rows per core
TH = Q + WINDOW  # rows incl. halo = 1024
EC = E // 128    # 16 e-chunks
EP = EC // 2     # 8 e-chunk pairs (DoubleRow contracts 256 at a time)
NQB = Q // 128   # 4 query blocks per core
NJ = 5           # key blocks per query block (window 512 + diag)
F32 = mybir.dt.float32
BF = mybir.dt.bfloat16
F8 = mybir.dt.float8e4
DR = mybir.MatmulPerfMode.DoubleRow
COPY = mybir.ActivationFunctionType.Copy

_CACHE = {}


def _build_bass():
    nc = bacc.Bacc("TRN2", target_bir_lowering=False, debug=False,
                   enable_asserts=True, num_devices=NCORES)

    xT_d = nc.dram_tensor("xT", [128, 2, EC, 512], BF, kind="ExternalInput")
    xt8_d = nc.dram_tensor("xt8", [128, 2, EC, 512], F8, kind="ExternalInput")
    wq_d = nc.dram_tensor("wq", [H, 128, EC, 128], F8, kind="ExternalInput")
    wk_d = nc.dram_tensor("wk", [128, KV, EC, 128], F8, kind="ExternalInput")
    wv_d = nc.dram_tensor("wv", [128, EC, KV * 128], BF, kind="ExternalInput")
    wo_d = nc.dram_tensor("wo", [4, 128, H, 512], BF, kind="ExternalInput")
    # rope tables stacked for full-128-partition DVE ops:
    # cos2 = [cos; cos], sin2 = [+sin; -sin]
    cosk_d = nc.dram_tensor("cos_k", [128, TH], F32, kind="ExternalInput")
    sink_d = nc.dram_tensor("sin_k", [128, TH], F32, kind="ExternalInput")
    cosq_d = nc.dram_tensor("cos_q", [128, Q], F32, kind="ExternalInput")
    sinq_d = nc.dram_tensor("sin_q", [128, Q], F32, kind="ExternalInput")
    m0_d = nc.dram_tensor("mask0", [128, 512], BF, kind="ExternalInput")
    m4_d = nc.dram_tensor("mask4", [128, 512], BF, kind="ExternalInput")
    corr_d = nc.dram_tensor("corr", [1, NQB * 512], F32, kind="ExternalInput")
    out_d = nc.dram_tensor("out", [Q, E], F32, kind="ExternalOutput")

    EXP = mybir.ActivationFunctionType.Exp

    with tile.TileContext(nc) as tc:
        with (
            tc.tile_pool(name="const", bufs=1) as const,
            tc.tile_pool(name="tmp", bufs=3) as tmp,
            tc.tile_pool(name="probs", bufs=9) as probsp,
            tc.tile_pool(name="accp", bufs=2) as accp,
            tc.tile_pool(name="small", bufs=3) as small,
            tc.tile_pool(name="bcp", bufs=3) as bcp,
            tc.tile_pool(name="attu", bufs=3) as attup,
            tc.tile_pool(name="outp", bufs=3) as outp,
            tc.tile_pool(name="vp", bufs=1) as vp,
            tc.tile_pool(name="ps_proj", bufs=2, space="PSUM") as ps_proj,
            tc.tile_pool(name="ps_sc", bufs=3, space="PSUM") as ps_scp,
            tc.tile_pool(name="ps_att", bufs=2, space="PSUM") as ps_attp,
            tc.tile_pool(name="ps_den", bufs=1, space="PSUM") as ps_denp,
        ):
            # ---- persistent tensors ----
            m0 = const.tile([128, 512], BF, name="m0")
            nc.sync.dma_start(out=m0, in_=m0_d[:, :])
            m4 = const.tile([128, 512], BF, name="m4")
            nc.sync.dma_start(out=m4, in_=m4_d[:, :])
            corr = const.tile([1, NQB * 512], F32, name="corr")
            nc.sync.dma_start(out=corr, in_=corr_d[:, :])
            zero_b = const.tile([128, 1], F32, name="zero_b")
            nc.vector.memset(zero_b, 0.0)
            ones_b = const.tile([128, 1], BF, name="ones_b")
            nc.vector.memset(ones_b, 1.0)

            kT = [const.tile([128, TH], BF, tag=f"kT{g}", name=f"kT{g}")
                  for g in range(KV)]
            v_sb = [const.tile([128, KV * 128], BF, tag=f"v{tv}", name=f"v{tv}")
                    for tv in range(TH // 128)]
            qT = [const.tile([128, NREP, Q], BF, tag=f"qT{g}", name=f"qT{g}")
                  for g in range(KV)]
            att_sb = {}
            for g in range(KV):
                for qb in range(NQB):
                    att_sb[(g, qb)] = const.tile(
                        [128, 512], BF, tag=f"at{g}_{qb}", name=f"at{g}_{qb}")

            def rope(dst, ps, cos2, sin2, n):
                """dst[:128, :n] (bf16) <- rope(ps[:128, :n] fp32).

                a = ps[0:64], b = ps[64:128]; sin2 = [-s; +s] stacked:
                  A          = ps * [c;c]              (DVE, 128p, PSUM in)
                  Bv[0:64]   = ps[64:128] * (-s)       (DVE, 64p, PSUM in)
                  Bv[64:128] = ps[0:64]   * (+s)       (DVE, 64p, PSUM in)
                  dst = A + Bv                         (GpSimd, 128p)
                PSUM reads are exempt from the equal-base-partition rule for
                two-SBUF-input TensorTensor ops, so the half swap happens in
                the multiplies; the add sees aligned SBUF tiles.
                """
                A = tmp.tile([128, n], F32, tag="ropeA", name="ropeA")
                nc.vector.tensor_mul(A, ps, cos2)
                Bv = tmp.tile([128, n], F32, tag="ropeB", name="ropeB")
                nc.vector.tensor_mul(Bv[0:64, :], ps[64:128, :], sin2[0:64, :])
                nc.vector.tensor_mul(Bv[64:128, :], ps[0:64, :], sin2[64:128, :])
                nc.gpsimd.tensor_add(dst, A, Bv)

            # ---- projection phase ----
            with (
                tc.tile_pool(name="xtp", bufs=1) as xtp,
                tc.tile_pool(name="wqp", bufs=2) as wqp,
            ):
                # smallest DMAs first: the first k-proj chain needs only
                # wk8[g0] (256KB) + xt8 th0 (1MB, split in two)
                wk8_t = xtp.tile([128, KV, EC, 128], F8, name="wk8")
                nc.sync.dma_start(out=wk8_t[:, 0], in_=wk_d[:, 0])
                xt8_t = xtp.tile([128, 2, EC, 512], F8, name="xt8")
                nc.sync.dma_start(out=xt8_t[:, 0, 0:8], in_=xt8_d[:, 0, 0:8])
                nc.sync.dma_start(out=xt8_t[:, 0, 8:16], in_=xt8_d[:, 0, 8:16])
                for g in range(1, KV):
                    nc.sync.dma_start(out=wk8_t[:, g], in_=wk_d[:, g])
                nc.sync.dma_start(out=xt8_t[:, 1], in_=xt8_d[:, 1])
                cosk = xtp.tile([128, TH], F32, name="cosk")
                nc.sync.dma_start(out=cosk, in_=cosk_d[:, :])
                sink = xtp.tile([128, TH], F32, name="sink")
                nc.sync.dma_start(out=sink, in_=sink_d[:, :])
                # bf16 x halves (for the v projection only); th1 lives in the
                # vp pool because v-proj blocks tv=5..7 are emitted inside the
                # attention phase to fill the qb0 tensor bubble
                xt_th0 = xtp.tile([128, EC, 512], BF, name="xt_th0")
                nc.sync.dma_start(out=xt_th0, in_=xT_d[:, 0])
                xt_th1 = vp.tile([128, EC, 512], BF, name="xt_th1")
                nc.sync.dma_start(out=xt_th1, in_=xT_d[:, 1])
                wv_t = vp.tile([128, EC, KV * 128], BF, name="wv_t")
                nc.sync.dma_start(out=wv_t, in_=wv_d[:, :, :])
                cosq = xtp.tile([128, Q], F32, name="cosq")
                nc.sync.dma_start(out=cosq, in_=cosq_d[:, :])
                sinq = xtp.tile([128, Q], F32, name="sinq")
                nc.sync.dma_start(out=sinq, in_=sinq_d[:, :])

                # k projection + rope, fp8 DoubleRow (K=256 per matmul)
                for th in range(2):
                    for g in range(KV):
                        sl = slice(th * 512, (th + 1) * 512)
                        ps = ps_proj.tile([128, 512], F32, tag="proj", name="psk")
                        for e in range(EP):
                            nc.tensor.matmul(
                                ps, wk8_t[:, g, 2 * e:2 * e + 2, :],
                                xt8_t[:, th, 2 * e:2 * e + 2, :],
                                start=(e == 0), stop=(e == EP - 1), perf_mode=DR)
                        rope(kT[g][:, sl], ps, cosk[:, sl], sink[:, sl], 512)

                def vproj(tv):
                    th, lo = tv // 4, (tv % 4) * 128
                    xh = xt_th0 if th == 0 else xt_th1
                    ps = ps_proj.tile([128, 512], F32, tag="proj", name="psv")
                    for ec in range(EC):
                        nc.tensor.matmul(ps, xh[:, ec, lo:lo + 128],
                                         wv_t[:, ec, :],
                                         start=(ec == 0), stop=(ec == EC - 1))
                    nc.scalar.activation(v_sb[tv], ps, COPY)

                # v projection blocks needed by attention qb0 (kb 0..4)
                for tv in range(5):
                    vproj(tv)

                # q projection + rope, fp8 DoubleRow (scale folded into rope)
                for g in range(KV):
                    for hg in range(NREP):
                        h = g * NREP + hg
                        wq_t = wqp.tile([128, EC, 128], F8, tag="wq", name="wq_t")
                        nc.sync.dma_start(out=wq_t, in_=wq_d[h, :, :, :])
                        ps = ps_proj.tile([128, 512], F32, tag="proj", name="psq")
                        for e in range(EP):
                            nc.tensor.matmul(
                                ps, wq_t[:, 2 * e:2 * e + 2, :],
                                xt8_t[:, 1, 2 * e:2 * e + 2, :],
                                start=(e == 0), stop=(e == EP - 1), perf_mode=DR)
                        rope(qT[g][:, hg, :], ps, cosq, sinq, Q)

            # ---- attention + output projection, interleaved ----
            with tc.tile_pool(name="wop", bufs=1) as wop:
                wo_t = []
                for ec in range(4):
                    w = wop.tile([128, H, 512], BF, name=f"wo{ec}")
                    nc.sync.dma_start(out=w, in_=wo_d[ec, :, :, :])
                    wo_t.append(w)

                def attention(qb, g):
                    rhs_q = qT[g][:, :, qb * 128:(qb + 1) * 128]
                    ps_att = ps_attp.tile([128, 512], F32, tag="att",
                                          name="ps_att")
                    prs = []

                    def emit_scores(j):
                        kb = qb + j
                        ksl = slice(kb * 128, (kb + 1) * 128)
                        ps_sc = ps_scp.tile([128, 512], F32, tag="sc",
                                            name="ps_sc")
                        nc.tensor.matmul(ps_sc, kT[g][:, ksl], rhs_q,
                                         start=True, stop=True)
                        pr = probsp.tile([128, 512], BF, tag="pr", name="pr")
                        nc.scalar.activation(pr, ps_sc, EXP, bias=zero_b[:, :])
                        if j == 0:
                            nc.vector.tensor_mul(pr, pr, m0)
                        elif j == NJ - 1:
                            nc.vector.tensor_mul(pr, pr, m4)
                        prs.append(pr)

                    def emit_av(j):
                        kb = qb + j
                        nc.tensor.matmul(
                            ps_att, v_sb[kb][:, g * 128:(g + 1) * 128],
                            prs[j], start=(j == 0), stop=(j == NJ - 1))

                    # scores run ahead of attn@v so the tensor engine never
                    # waits on ScalarE's exp
                    emit_scores(0)
                    emit_scores(1)
                    emit_scores(2)
                    emit_av(0)
                    emit_scores(3)
                    emit_av(1)
                    emit_scores(4)
                    emit_av(2)
                    emit_av(3)
                    emit_av(4)

                    # denominator: sum the 5 prob blocks on DVE (bf16, 2x
                    # rate), then one ones-vector matmul
                    acc = accp.tile([128, 512], BF, tag="acc", name="acc")
                    nc.vector.tensor_add(acc, prs[0], prs[1])
                    nc.vector.tensor_add(acc, acc, prs[2])
                    nc.vector.tensor_add(acc, acc, prs[3])
                    nc.vector.tensor_add(acc, acc, prs[4])
                    ps_den = ps_denp.tile([1, 512], F32, tag="den",
                                          name="ps_den")
                    nc.tensor.matmul(ps_den, ones_b, acc, start=True, stop=True)

                    # free the ps_att slot right away (ScalarE copy);
                    # normalize off the critical path once the reciprocal
                    # broadcast lands
                    att_un = attup.tile([128, 512], F32, tag="attu",
                                        name="att_un")
                    nc.scalar.activation(att_un, ps_att, COPY)
                    den_s = small.tile([1, 512], F32, tag="den_s", name="den_s")
                    nc.vector.tensor_sub(den_s, ps_den,
                                         corr[:, qb * 512:(qb + 1) * 512])
                    # full-precision DVE reciprocal on a 1-partition tile
                    # costs 3.3us; the 18-bit approx is ~5x faster and far
                    # below the bf16 noise floor
                    rec = small.tile([1, 512], F32, tag="rec", name="rec")
                    nc.vector.reciprocal_approx_fast(out=rec, in_=den_s)
                    bc_sb = bcp.tile([128, 512], F32, tag="bcs", name="bc_sb")
                    nc.gpsimd.partition_broadcast(bc_sb, rec)
                    nc.vector.tensor_mul(att_sb[(g, qb)], att_un, bc_sb)

                def oproj_ec(qb, ec):
                    ps = ps_proj.tile([128, 512], F32, tag="proj", name="pso")
                    for h in range(H):
                        g, hg = h // NREP, h % NREP
                        nc.tensor.matmul(
                            ps, att_sb[(g, qb)][:, hg * 128:(hg + 1) * 128],
                            wo_t[ec][:, h, :], start=(h == 0),
                            stop=(h == H - 1))
                    # evacuate PSUM on ScalarE (it has slack; DVE does not),
                    # then DMA the 256KB chunk right away
                    ob = outp.tile([128, 512], F32, tag="ob", name="ob")
                    nc.scalar.activation(ob, ps, COPY)
                    nc.sync.dma_start(
                        out=out_d[qb * 128:(qb + 1) * 128,
                                  ec * 512:(ec + 1) * 512],
                        in_=ob)

                for qb in range(NQB):
                    for g in range(KV):
                        attention(qb, g)
                        if qb >= 1:
                            oproj_ec(qb - 1, g)
                        elif g >= 1:
                            # fill the qb0 bubble (no oproj ready yet) with
                            # the remaining v-projection blocks
                            vproj(4 + g)

                for g in range(KV):
                    oproj_ec(NQB - 1, g)

    nc.compile()
    return nc


def _prep_inputs(x, Wq, Wk, Wv, Wo):
    """Host-side prep: shard + transpose + cast. Returns list of in_maps."""
    x = np.asarray(x, np.float32)
    Wq = np.asarray(Wq, np.float32)
    Wk = np.asarray(Wk, np.float32)
    Wv = np.asarray(Wv, np.float32)
    Wo = np.asarray(Wo, np.float32)

    def to_fp8(a):
        return np.clip(a, -240.0, 240.0).astype(FP8)

    # weights: shared across cores
    # wq[h, e_in, ec, hd] = 128*Wq[h*128+hd, ec*128+e_in]  (fp8)
    wq = to_fp8(np.ascontiguousarray(
        Wq.reshape(H, 128, EC, 128).transpose(0, 3, 2, 1)) * W_SCALE)
    # wk[e_in, g, ec, d] = 128*Wk[g*128+d, ec*128+e_in]  (fp8)
    wk = to_fp8(np.ascontiguousarray(
        Wk.reshape(KV, 128, EC, 128).transpose(3, 0, 2, 1)) * W_SCALE)
    # wv[e_in, ec, gd] = Wv[gd, ec*128+e_in]
    wv = np.ascontiguousarray(
        Wv.reshape(KV * 128, EC, 128).transpose(2, 1, 0)).astype(BF16)
    # wo[ec, d, h, e] = Wo[ec*512+e, h*128+d]
    wo = np.ascontiguousarray(
        Wo.reshape(4, 512, H, 128).transpose(0, 3, 2, 1)).astype(BF16)

    inv_freq = 1.0 / (THETA ** (np.arange(0, D, 2, dtype=np.float32) / D))  # [64]
    scale = np.float32(1.0 / np.sqrt(D))
    descale = np.float32(1.0 / W_SCALE)

    # masks (tiled over the 4 heads of a group along the free dim)
    kp = np.arange(128)[:, None]
    qf = np.arange(128)[None, :]
    m0 = np.tile((kp > qf).astype(np.float32), (1, NREP)).astype(BF16)
    m4 = np.tile((kp <= qf).astype(np.float32), (1, NREP)).astype(BF16)

    def stack2(cos, sin):
        """[64,n] cos/sin -> [128,n] stacked [c;c], [-s;+s] (fp32)."""
        c2 = np.concatenate([cos, cos], 0).astype(np.float32)
        s2 = np.concatenate([-sin, sin], 0).astype(np.float32)
        return np.ascontiguousarray(c2), np.ascontiguousarray(s2)

    in_maps = []
    for c in range(NCORES):
        b, ch = c // 4, c % 4
        q0 = ch * Q
        # x with halo, zero-padded at sequence start
        xc = np.zeros((TH, E), np.float32)
        lo = q0 - WINDOW
        xc[max(0, -lo):] = x[b, max(0, lo):q0 + Q]
        # xT[p, th, ec, s] = xc[th*512+s, ec*128+p]
        xTr = np.ascontiguousarray(
            xc.reshape(2, 512, EC, 128).transpose(3, 0, 2, 1))
        xT = xTr.astype(BF16)
        xt8 = to_fp8(xTr)

        pos_k = np.arange(lo, q0 + Q, dtype=np.float32)
        ang_k = inv_freq[:, None] * pos_k[None, :]
        pos_q = np.arange(q0, q0 + Q, dtype=np.float32)
        ang_q = inv_freq[:, None] * pos_q[None, :]
        cos_k, sin_k = stack2(np.cos(ang_k) * descale, np.sin(ang_k) * descale)
        cos_q, sin_q = stack2(np.cos(ang_q) * scale * descale,
                              np.sin(ang_q) * scale * descale)

        # denominator correction: padded keys inside the window contribute
        # exp(0) = 1 each (only for sequence-start chunks)
        if ch == 0:
            q_l = WINDOW + np.arange(Q)
            cnt = np.maximum(0, (TH - 1) - q_l).astype(np.float32)  # 1023 - q_l
        else:
            cnt = np.zeros(Q, np.float32)
        corr = np.ascontiguousarray(
            np.tile(cnt.reshape(NQB, 1, 128), (1, NREP, 1)).reshape(1, NQB * 512))

        in_maps.append({
            "xT": xT, "xt8": xt8,
            "wq": wq, "wk": wk, "wv": wv, "wo": wo,
            "cos_k": cos_k, "sin_k": sin_k,
            "cos_q": cos_q, "sin_q": sin_q,
            "mask0": m0, "mask4": m4,
            "corr": corr,
        })
    return in_maps


def _get_nc():
    if "nc" not in _CACHE:
        _CACHE["nc"] = _build_bass()
    return _CACHE["nc"]


def run(inputs, trace=False, **kw):
    nc = _get_nc()
    in_maps = _prep_inputs(**inputs)
    res = run_bass_kernel_spmd(nc, in_maps, core_ids=list(range(NCORES)),
                               trace=trace, **kw)
    out = np.empty((B, T, E), np.float32)
    for c in range(NCORES):
        b, ch = c // 4, c % 4
        out[b, ch * Q:(ch + 1) * Q] = res.results[c]["out"]
    return out, res


def kernel(**inputs):
    out, _ = run(inputs, trace=False)
    return out


# revision 16
# speedup vs baseline: 1.2820x; 1.2820x over previous
"""Sliding-window GQA self-attention (B=2,T=2048,E=2048,H=16,KV=4,D=128,W=512)
on 8 Trainium2 NeuronCores.

Sharding: sequence-parallel. Core c owns 512 query rows (batch c//4, quarter
c%4) and receives a 512-row key/value halo (zero-padded before the sequence
start; padded keys contribute exactly exp(0)=1 to the softmax denominator,
which is subtracted out via a precomputed correction vector).

On-chip dataflow (per core):
  q/k projections run in fp8e4 with DoubleRow perf mode (K=256 per matmul,
  measured ~1.44x bf16); weights are pre-scaled by 128 into fp8 range and the
  descale is folded into the fp32 RoPE tables (along with 1/sqrt(D) for q).
  The fp8 quantization noise is strongly damped by the near-uniform softmax
  (scores std ~0.023): measured cost ~1e-3 rel err. v projection and
  everything downstream stays bf16 (fp8 there costs 2e-2+).

  DVE ops cost ~550ns each regardless of size, so the kernel minimizes DVE
  op count and spreads elementwise work across engines: RoPE is 2 DVE
  multiplies on all 128 partitions (stacked [cos;cos], [+sin;-sin] tables)
  plus 2 GpSimd adds that un-swap the halves via partition-offset reads;
  PSUM evacuations ride ScalarE; the denominator correction subtract rides
  GpSimd. Scores are computed transposed (scoresT[k, q], 4 query heads of a
  kv group batched into one N=512 matmul) -> exp on ScalarE (no max
  subtraction: |scores| < 0.12) -> sliding-window masks applied
  multiplicatively post-exp. Softmax denominator: the 5 prob blocks per
  (group, query-block) are summed with 4 bf16 DVE adds, then a single
  ones-vector matmul. Emission order keeps the tensor engine fed: scores run
  ahead of attn@v, the last 3 v-projection blocks fill the attention-qb0
  window, and the output projection of query block qb-1 interleaves
  per-group with the attention of qb.
"""

import numpy as np
import ml_dtypes

import concourse.bass as bass
import concourse.bacc as bacc
import concourse.mybir as mybir
import concourse.tile as tile
from concourse.bass_utils import run_bass_kernel_spmd

BF16 = ml_dtypes.bfloat16
FP8 = ml_dtypes.float8_e4m3fn

B, T, E = 2, 2048, 2048
H, KV, D = 16, 4, 128
NREP = H // KV  # 4 query heads per kv head
WINDOW = 512
THETA = 10000.0
W_SCALE = 128.0  # fp8 pre-scale on Wq/Wk; descale folded into rope tables

NCORES = 8
Q = 512          # owned query rows per core
TH = Q + WINDOW  # rows incl. halo = 1024
EC = E // 128    # 16 e-chunks
EP = EC // 2     # 8 e-chunk pairs (DoubleRow contracts 256 at a time)
NQB = Q // 128   # 4 query blocks per core
NJ = 5           # key blocks per query block (window 512 + diag)
F32 = mybir.dt.float32
BF = mybir.dt.bfloat16
F8 = mybir.dt.float8e4
DR = mybir.MatmulPerfMode.DoubleRow
COPY = mybir.ActivationFunctionType.Copy

_CACHE = {}


def _build_bass():
    nc = bacc.Bacc("TRN2", target_bir_lowering=False, debug=False,
                   enable_asserts=True, num_devices=NCORES)

    xT_d = nc.dram_tensor("xT", [128, 2, EC, 512], BF, kind="ExternalInput")
    xt8_d = nc.dram_tensor("xt8", [128, 2, EC, 512], F8, kind="ExternalInput")
    wq_d = nc.dram_tensor("wq", [H, 128, EC, 128], F8, kind="ExternalInput")
    wk_d = nc.dram_tensor("wk", [128, KV, EC, 128], F8, kind="ExternalInput")
    wv_d = nc.dram_tensor("wv", [128, EC, KV * 128], BF, kind="ExternalInput")
    wo_d = nc.dram_tensor("wo", [4, 128, H, 512], BF, kind="ExternalInput")
    # rope tables stacked for full-128-partition DVE ops:
    # cos2 = [cos; cos], sin2 = [+sin; -sin]
    cosk_d = nc.dram_tensor("cos_k", [128, TH], F32, kind="ExternalInput")
    sink_d = nc.dram_tensor("sin_k", [128, TH], F32, kind="ExternalInput")
    cosq_d = nc.dram_tensor("cos_q", [128, Q], F32, kind="ExternalInput")
    sinq_d = nc.dram_tensor("sin_q", [128, Q], F32, kind="ExternalInput")
    m0_d = nc.dram_tensor("mask0", [128, 512], BF, kind="ExternalInput")
    m4_d = nc.dram_tensor("mask4", [128, 512], BF, kind="ExternalInput")
    corr_d = nc.dram_tensor("corr", [1, NQB * 512], F32, kind="ExternalInput")
    out_d = nc.dram_tensor("out", [Q, E], F32, kind="ExternalOutput")

    EXP = mybir.ActivationFunctionType.Exp

    with tile.TileContext(nc) as tc:
        with (
            tc.tile_pool(name="const", bufs=1) as const,
            tc.tile_pool(name="tmp", bufs=3) as tmp,
            tc.tile_pool(name="probs", bufs=9) as probsp,
            tc.tile_pool(name="accp", bufs=2) as accp,
            tc.tile_pool(name="small", bufs=3) as small,
            tc.tile_pool(name="bcp", bufs=3) as bcp,
            tc.tile_pool(name="attu", bufs=3) as attup,
            tc.tile_pool(name="outp", bufs=3) as outp,
            tc.tile_pool(name="vp", bufs=1) as vp,
            tc.tile_pool(name="ps_proj", bufs=2, space="PSUM") as ps_proj,
            tc.tile_pool(name="ps_sc", bufs=3, space="PSUM") as ps_scp,
            tc.tile_pool(name="ps_att", bufs=2, space="PSUM") as ps_attp,
            tc.tile_pool(name="ps_den", bufs=1, space="PSUM") as ps_denp,
        ):
            # ---- persistent tensors ----
            m0 = const.tile([128, 512], BF, name="m0")
            nc.sync.dma_start(out=m0, in_=m0_d[:, :])
            m4 = const.tile([128, 512], BF, name="m4")
            nc.sync.dma_start(out=m4, in_=m4_d[:, :])
            corr = const.tile([1, NQB * 512], F32, name="corr")
            nc.sync.dma_start(out=corr, in_=corr_d[:, :])
            zero_b = const.tile([128, 1], F32, name="zero_b")
            nc.vector.memset(zero_b, 0.0)
            ones_b = const.tile([128, 1], BF, name="ones_b")
            nc.vector.memset(ones_b, 1.0)

            kT = [const.tile([128, TH], BF, tag=f"kT{g}", name=f"kT{g}")
                  for g in range(KV)]
            v_sb = [const.tile([128, KV * 128], BF, tag=f"v{tv}", name=f"v{tv}")
                    for tv in range(TH // 128)]
            qT = [const.tile([128, NREP, Q], BF, tag=f"qT{g}", name=f"qT{g}")
                  for g in range(KV)]
            att_sb = {}
            for g in range(KV):
                for qb in range(NQB):
                    att_sb[(g, qb)] = const.tile(
                        [128, 512], BF, tag=f"at{g}_{qb}", name=f"at{g}_{qb}")

            def rope(dst, ps, cos2, sin2, n):
                """dst[:128, :n] (bf16) <- rope(ps[:128, :n] fp32).

                a = ps[0:64], b = ps[64:128]; sin2 = [-s; +s] stacked:
                  A          = ps * [c;c]              (DVE, 128p, PSUM in)
                  Bv[0:64]   = ps[64:128] * (-s)       (DVE, 64p, PSUM in)
                  Bv[64:128] = ps[0:64]   * (+s)       (DVE, 64p, PSUM in)
                  dst = A + Bv                         (DVE, 128p)
                PSUM reads are exempt from the equal-base-partition rule for
                two-SBUF-input TensorTensor ops, so the half swap happens in
                the multiplies; the add sees aligned SBUF tiles. 4 DVE ops
                instead of the rotate-half-native 6.
                """
                A = tmp.tile([128, n], F32, tag="ropeA", name="ropeA")
                nc.vector.tensor_mul(A, ps, cos2)
                Bv = tmp.tile([128, n], F32, tag="ropeB", name="ropeB")
                nc.vector.tensor_mul(Bv[0:64, :], ps[64:128, :], sin2[0:64, :])
                nc.vector.tensor_mul(Bv[64:128, :], ps[0:64, :], sin2[64:128, :])
                nc.vector.tensor_add(dst, A, Bv)

            # ---- projection phase ----
            with (
                tc.tile_pool(name="xtp", bufs=1) as xtp,
                tc.tile_pool(name="wqp", bufs=2) as wqp,
            ):
                # Allocate xtp tiles with the EARLY-DYING ones first (wk8,
                # cos/sin k-tables, xt_th0): the wo weight tiles allocated
                # after this pool closes reuse its SBUF region from the
                # start, so their DMAs only wait on tiles that die early.
                wk8_t = xtp.tile([128, KV, EC, 128], F8, name="wk8")
                cosk = xtp.tile([128, TH], F32, name="cosk")
                sink = xtp.tile([128, TH], F32, name="sink")
                xt_th0 = xtp.tile([128, EC, 512], BF, name="xt_th0")
                xt8_t = xtp.tile([128, 2, EC, 512], F8, name="xt8")
                cosq = xtp.tile([128, Q], F32, name="cosq")
                sinq = xtp.tile([128, Q], F32, name="sinq")
                xt_th1 = vp.tile([128, EC, 512], BF, name="xt_th1")
                wv_t = vp.tile([128, EC, KV * 128], BF, name="wv_t")

                # smallest DMAs first: the first k-proj chain needs only
                # wk8[g0] (256KB) + xt8 th0 (1MB, split in two); the k rope
                # tables must beat the first rope call, so they go before
                # the remaining wk groups
                nc.sync.dma_start(out=wk8_t[:, 0], in_=wk_d[:, 0])
                nc.sync.dma_start(out=xt8_t[:, 0, 0:8], in_=xt8_d[:, 0, 0:8])
                nc.sync.dma_start(out=xt8_t[:, 0, 8:16], in_=xt8_d[:, 0, 8:16])
                nc.sync.dma_start(out=cosk, in_=cosk_d[:, :])
                nc.sync.dma_start(out=sink, in_=sink_d[:, :])
                for g in range(1, KV):
                    nc.sync.dma_start(out=wk8_t[:, g], in_=wk_d[:, g])
                nc.sync.dma_start(out=xt8_t[:, 1], in_=xt8_d[:, 1])
                nc.sync.dma_start(out=xt_th0, in_=xT_d[:, 0])
                nc.sync.dma_start(out=xt_th1, in_=xT_d[:, 1])
                nc.sync.dma_start(out=wv_t, in_=wv_d[:, :, :])
                nc.sync.dma_start(out=cosq, in_=cosq_d[:, :])
                nc.sync.dma_start(out=sinq, in_=sinq_d[:, :])

                # k projection + rope, fp8 DoubleRow (K=256 per matmul)
                for th in range(2):
                    for g in range(KV):
                        sl = slice(th * 512, (th + 1) * 512)
                        ps = ps_proj.tile([128, 512], F32, tag="proj", name="psk")
                        for e in range(EP):
                            nc.tensor.matmul(
                                ps, wk8_t[:, g, 2 * e:2 * e + 2, :],
                                xt8_t[:, th, 2 * e:2 * e + 2, :],
                                start=(e == 0), stop=(e == EP - 1), perf_mode=DR)
                        rope(kT[g][:, sl], ps, cosk[:, sl], sink[:, sl], 512)

                def vproj(tv):
                    th, lo = tv // 4, (tv % 4) * 128
                    xh = xt_th0 if th == 0 else xt_th1
                    ps = ps_proj.tile([128, 512], F32, tag="proj", name="psv")
                    for ec in range(EC):
                        nc.tensor.matmul(ps, xh[:, ec, lo:lo + 128],
                                         wv_t[:, ec, :],
                                         start=(ec == 0), stop=(ec == EC - 1))
                    nc.scalar.activation(v_sb[tv], ps, COPY)

                # v projection blocks needed by attention qb0 (kb 0..4)
                for tv in range(5):
                    vproj(tv)

                # q projection + rope, fp8 DoubleRow (scale folded into rope)
                for g in range(KV):
                    for hg in range(NREP):
                        h = g * NREP + hg
                        wq_t = wqp.tile([128, EC, 128], F8, tag="wq", name="wq_t")
                        nc.sync.dma_start(out=wq_t, in_=wq_d[h, :, :, :])
                        ps = ps_proj.tile([128, 512], F32, tag="proj", name="psq")
                        for e in range(EP):
                            nc.tensor.matmul(
                                ps, wq_t[:, 2 * e:2 * e + 2, :],
                                xt8_t[:, 1, 2 * e:2 * e + 2, :],
                                start=(e == 0), stop=(e == EP - 1), perf_mode=DR)
                        rope(qT[g][:, hg, :], ps, cosq, sinq, Q)

            # ---- attention + output projection, interleaved ----
            with tc.tile_pool(name="wop", bufs=1) as wop:
                wo_t = []
                for ec in range(4):
                    w = wop.tile([128, H, 512], BF, name=f"wo{ec}")
                    nc.sync.dma_start(out=w, in_=wo_d[ec, :, :, :])
                    wo_t.append(w)

                def attention(qb, g):
                    rhs_q = qT[g][:, :, qb * 128:(qb + 1) * 128]
                    ps_att = ps_attp.tile([128, 512], F32, tag="att",
                                          name="ps_att")
                    prs = []

                    def emit_scores(j):
                        kb = qb + j
                        ksl = slice(kb * 128, (kb + 1) * 128)
                        ps_sc = ps_scp.tile([128, 512], F32, tag="sc",
                                            name="ps_sc")
                        nc.tensor.matmul(ps_sc, kT[g][:, ksl], rhs_q,
                                         start=True, stop=True)
                        pr = probsp.tile([128, 512], BF, tag="pr", name="pr")
                        nc.scalar.activation(pr, ps_sc, EXP, bias=zero_b[:, :])
                        if j == 0:
                            nc.vector.tensor_mul(pr, pr, m0)
                        elif j == NJ - 1:
                            nc.vector.tensor_mul(pr, pr, m4)
                        prs.append(pr)

                    def emit_av(j):
                        kb = qb + j
                        nc.tensor.matmul(
                            ps_att, v_sb[kb][:, g * 128:(g + 1) * 128],
                            prs[j], start=(j == 0), stop=(j == NJ - 1))

                    # scores run ahead of attn@v so the tensor engine never
                    # waits on ScalarE's exp
                    emit_scores(0)
                    emit_scores(1)
                    emit_scores(2)
                    emit_av(0)
                    emit_scores(3)
                    emit_av(1)
                    emit_scores(4)
                    emit_av(2)
                    emit_av(3)
                    emit_av(4)

                    # denominator: sum the 5 prob blocks on DVE (bf16, 2x
                    # rate), then one ones-vector matmul
                    acc = accp.tile([128, 512], BF, tag="acc", name="acc")
                    nc.vector.tensor_add(acc, prs[0], prs[1])
                    nc.vector.tensor_add(acc, acc, prs[2])
                    nc.vector.tensor_add(acc, acc, prs[3])
                    nc.vector.tensor_add(acc, acc, prs[4])
                    ps_den = ps_denp.tile([1, 512], F32, tag="den",
                                          name="ps_den")
                    nc.tensor.matmul(ps_den, ones_b, acc, start=True, stop=True)

                    # free the ps_att slot right away (ScalarE copy);
                    # normalize off the critical path once the reciprocal
                    # broadcast lands
                    att_un = attup.tile([128, 512], F32, tag="attu",
                                        name="att_un")
                    nc.scalar.activation(att_un, ps_att, COPY)
                    den_s = small.tile([1, 512], F32, tag="den_s", name="den_s")
                    nc.vector.tensor_sub(den_s, ps_den,
                                         corr[:, qb * 512:(qb + 1) * 512])
                    # full-precision DVE reciprocal on a 1-partition tile
                    # costs 3.3us; the 18-bit approx is ~5x faster and far
                    # below the bf16 noise floor
                    rec = small.tile([1, 512], F32, tag="rec", name="rec")
                    nc.vector.reciprocal_approx_fast(out=rec, in_=den_s)
                    bc_sb = bcp.tile([128, 512], F32, tag="bcs", name="bc_sb")
                    nc.gpsimd.partition_broadcast(bc_sb, rec)
                    nc.vector.tensor_mul(att_sb[(g, qb)], att_un, bc_sb)

                def oproj_ec(qb, ec):
                    ps = ps_proj.tile([128, 512], F32, tag="proj", name="pso")
                    for h in range(H):
                        g, hg = h // NREP, h % NREP
                        nc.tensor.matmul(
                            ps, att_sb[(g, qb)][:, hg * 128:(hg + 1) * 128],
                            wo_t[ec][:, h, :], start=(h == 0),
                            stop=(h == H - 1))
                    # evacuate PSUM on ScalarE (it has slack; DVE does not),
                    # then DMA the 256KB chunk right away
                    ob = outp.tile([128, 512], F32, tag="ob", name="ob")
                    nc.scalar.activation(ob, ps, COPY)
                    nc.sync.dma_start(
                        out=out_d[qb * 128:(qb + 1) * 128,
                                  ec * 512:(ec + 1) * 512],
                        in_=ob)

                for qb in range(NQB):
                    for g in range(KV):
                        attention(qb, g)
                        if qb >= 1:
                            oproj_ec(qb - 1, g)
                        elif g >= 1:
                            # fill the qb0 bubble (no oproj ready yet) with
                            # the remaining v-projection blocks
                            vproj(4 + g)

                for g in range(KV):
                    oproj_ec(NQB - 1, g)

    nc.compile()
    return nc


def _prep_inputs(x, Wq, Wk, Wv, Wo):
    """Host-side prep: shard + transpose + cast. Returns list of in_maps."""
    x = np.asarray(x, np.float32)
    Wq = np.asarray(Wq, np.float32)
    Wk = np.asarray(Wk, np.float32)
    Wv = np.asarray(Wv, np.float32)
    Wo = np.asarray(Wo, np.float32)

    def to_fp8(a):
        return np.clip(a, -240.0, 240.0).astype(FP8)

    # weights: shared across cores
    # wq[h, e_in, ec, hd] = 128*Wq[h*128+hd, ec*128+e_in]  (fp8)
    wq = to_fp8(np.ascontiguousarray(
        Wq.reshape(H, 128, EC, 128).transpose(0, 3, 2, 1)) * W_SCALE)
    # wk[e_in, g, ec, d] = 128*Wk[g*128+d, ec*128+e_in]  (fp8)
    wk = to_fp8(np.ascontiguousarray(
        Wk.reshape(KV, 128, EC, 128).transpose(3, 0, 2, 1)) * W_SCALE)
    # wv[e_in, ec, gd] = Wv[gd, ec*128+e_in]
    wv = np.ascontiguousarray(
        Wv.reshape(KV * 128, EC, 128).transpose(2, 1, 0)).astype(BF16)
    # wo[ec, d, h, e] = Wo[ec*512+e, h*128+d]
    wo = np.ascontiguousarray(
        Wo.reshape(4, 512, H, 128).transpose(0, 3, 2, 1)).astype(BF16)

    inv_freq = 1.0 / (THETA ** (np.arange(0, D, 2, dtype=np.float32) / D))  # [64]
    scale = np.float32(1.0 / np.sqrt(D))
    descale = np.float32(1.0 / W_SCALE)

    # masks (tiled over the 4 heads of a group along the free dim)
    kp = np.arange(128)[:, None]
    qf = np.arange(128)[None, :]
    m0 = np.tile((kp > qf).astype(np.float32), (1, NREP)).astype(BF16)
    m4 = np.tile((kp <= qf).astype(np.float32), (1, NREP)).astype(BF16)

    def stack2(cos, sin):
        """[64,n] cos/sin -> [128,n] stacked [c;c], [-s;+s] (fp32)."""
        c2 = np.concatenate([cos, cos], 0).astype(np.float32)
        s2 = np.concatenate([-sin, sin], 0).astype(np.float32)
        return np.ascontiguousarray(c2), np.ascontiguousarray(s2)

    in_maps = []
    for c in range(NCORES):
        b, ch = c // 4, c % 4
        q0 = ch * Q
        # x with halo, zero-padded at sequence start
        xc = np.zeros((TH, E), np.float32)
        lo = q0 - WINDOW
        xc[max(0, -lo):] = x[b, max(0, lo):q0 + Q]
        # xT[p, th, ec, s] = xc[th*512+s, ec*128+p]
        xTr = np.ascontiguousarray(
            xc.reshape(2, 512, EC, 128).transpose(3, 0, 2, 1))
        xT = xTr.astype(BF16)
        xt8 = to_fp8(xTr)

        pos_k = np.arange(lo, q0 + Q, dtype=np.float32)
        ang_k = inv_freq[:, None] * pos_k[None, :]
        pos_q = np.arange(q0, q0 + Q, dtype=np.float32)
        ang_q = inv_freq[:, None] * pos_q[None, :]
        cos_k, sin_k = stack2(np.cos(ang_k) * descale, np.sin(ang_k) * descale)
        cos_q, sin_q = stack2(np.cos(ang_q) * scale * descale,
                              np.sin(ang_q) * scale * descale)

        # denominator correction: padded keys inside the window contribute
        # exp(0) = 1 each (only for sequence-start chunks)
        if ch == 0:
            q_l = WINDOW + np.arange(Q)
            cnt = np.maximum(0, (TH - 1) - q_l).astype(np.float32)  # 1023 - q_l
        else:
            cnt = np.zeros(Q, np.float32)
        corr = np.ascontiguousarray(
            np.tile(cnt.reshape(NQB, 1, 128), (1, NREP, 1)).reshape(1, NQB * 512))

        in_maps.append({
            "xT": xT, "xt8": xt8,
            "wq": wq, "wk": wk, "wv": wv, "wo": wo,
            "cos_k": cos_k, "sin_k": sin_k,
            "cos_q": cos_q, "sin_q": sin_q,
            "mask0": m0, "mask4": m4,
            "corr": corr,
        })
    return in_maps


def _get_nc():
    if "nc" not in _CACHE:
        _CACHE["nc"] = _build_bass()
    return _CACHE["nc"]


def run(inputs, trace=False, **kw):
    nc = _get_nc()
    in_maps = _prep_inputs(**inputs)
    res = run_bass_kernel_spmd(nc, in_maps, core_ids=list(range(NCORES)),
                               trace=trace, **kw)
    out = np.empty((B, T, E), np.float32)
    for c in range(NCORES):
        b, ch = c // 4, c % 4
        out[b, ch * Q:(ch + 1) * Q] = res.results[c]["out"]
    return out, res


def kernel(**inputs):
    out, _ = run(inputs, trace=False)
    return out
